# revision 31
# baseline (speedup 1.0000x reference)
"""Trainium2 Bass kernel for nn_GCAModel (2D ST-LSTM recurrence + classifier).

Strategy (per the batch-data-parallel hint + anti-diagonal wavefront):
  - Shard batch B=128 across 8 cores (16 rows each); weights replicated.
  - Within a core, process the (t, j) grid along anti-diagonals d = t + j.
    All cells on a diagonal are independent -> one fused GEMM per diagonal.
  - The 16 batch rows are further split into TWO independent 8-row streams,
    interleaved diagonal by diagonal. The streams share no state, so while
    stream A's serial tail (sigmoid -> cell-update -> tanh -> h) ping-pongs
    between ScalarE and VectorE, stream B's GEMMs keep the tensor engine
    busy, and B's activations queue behind A's without head-of-line stalls.
  - Per stream, state h/c lives in SBUF as (128 part = H-chunk, 2 chunks,
    slot*8 cols); slot s holds row t = s-1 (slot 0 is a permanent zero
    guard). The GEMM for a diagonal starting at row t0 reads the h_t operand
    at slot offset t0 and the h_s operand at t0+1 -- the same buffer
    shifted by one slot.
  - Gates computed as g.T: out (gate-chunk 128, N<=200) = W.T @ h-chunk,
    accumulating 5 K-groups in ONE PSUM bank per gate (x+bias K=4, Wth
    2x128, Wsh 2x128). ScalarE applies sigmoid/tanh (PSUM->SBUF), VectorE
    does the cell update in place.
  - Mean-pool h via wide accumulators (fp16 partials, exact-ish for <=25
    adds; folded in fp32 at the end); classifier + log_softmax on device;
    (16, 60) per core, host concat.

Numerics: fp16 storage for x/weights/h/gates (PSUM accumulates fp32), bf16
cell state c (c grows beyond fp16 range). Measured 3.6e-3 absmax / 7.7e-4
rel err vs an fp64 oracle.
"""
import os
os.environ.setdefault("JAX_PLATFORMS", "axon,cpu")

import numpy as np

import concourse.bass as bass
import concourse.tile as tile
from concourse import bacc, mybir
from concourse.bass_utils import run_bass_kernel_spmd

# ---------------------------------------------------------------- problem dims
T, J, B, I, H, C = 100, 25, 128, 3, 256, 60
NCORES = 8
BL = B // NCORES            # 16 batch rows per core
NST = 2                     # independent streams per core
SB = BL // NST              # 8 batch rows per stream
G5 = 5 * H                  # 1280 gate columns
SLOTS = T + 1               # +1 zero-guard slot at the front
SW = SLOTS * SB             # per-stream state width per H-chunk (808)
NMAX = min(T, J) * SB       # widest per-stream diagonal: 25*8 = 200 cols

# gate order: u first, o last (c needs i,fs,ft,u; h needs o + tanh(c))
GATES = [("u", 4 * H, "Tanh"), ("i", 0, "Sigmoid"), ("fs", H, "Sigmoid"),
         ("ft", 2 * H, "Sigmoid"), ("o", 3 * H, "Sigmoid")]

# diagonals: d = t + j; per diagonal the active rows are [tlo, tlo+nd-1]
DIAGS = []
_off = 0
for _d in range(T + J - 1):
    _tlo, _thi = max(0, _d - (J - 1)), min(_d, T - 1)
    _nd = _thi - _tlo + 1
    DIAGS.append((_d, _tlo, _nd, _off))
    _off += _nd * SB
XSTRIDE = _off              # 20000 cols per stream
XCOLS = XSTRIDE * NST       # 40000

# ---------------------------------------------------------------- dtype knobs
MM_DT = mybir.dt.float16      # x / W / h storage (matmul operands)
GATE_DT = mybir.dt.float16    # post-activation gates, t1, tanh(c)
C_DT = mybir.dt.bfloat16      # cell state + c-temps (c exceeds fp16 range)
HS_DT = mybir.dt.float16      # per-slot h partial sums (<=25 adds, then f32)
MM_NP = np.float16

F32 = mybir.dt.float32


def _build_nc(reps=1):
    nc = bacc.Bacc("TRN2", target_bir_lowering=False, debug=False,
                   num_devices=NCORES)
    x_d = nc.dram_tensor("xdiag", [4, XCOLS], MM_DT, kind="ExternalInput")
    wih_d = nc.dram_tensor("wih", [4, G5], MM_DT, kind="ExternalInput")
    wth_d = nc.dram_tensor("wth", [128, 2, G5], MM_DT, kind="ExternalInput")
    wsh_d = nc.dram_tensor("wsh", [128, 2, G5], MM_DT, kind="ExternalInput")
    wc_d = nc.dram_tensor("wc", [128, 2, C], F32, kind="ExternalInput")
    bc_d = nc.dram_tensor("bc", [1, C], F32, kind="ExternalInput")
    out_d = nc.dram_tensor("out", [BL, C], F32, kind="ExternalOutput")

    AF = mybir.ActivationFunctionType

    with tile.TileContext(nc) as tc:
        with tc.tile_pool(name="const", bufs=1) as const, \
             tc.tile_pool(name="state", bufs=1) as state, \
             tc.tile_pool(name="gate", bufs=4) as gatep, \
             tc.tile_pool(name="work", bufs=4) as work, \
             tc.tile_pool(name="psg", bufs=7, space="PSUM") as psg, \
             tc.tile_pool(name="pscls", bufs=1, space="PSUM") as pscls:

            # ---- load constants
            wih_s = const.tile([4, G5], MM_DT)
            nc.sync.dma_start(out=wih_s, in_=wih_d[:, :])
            xs_all = const.tile([4, XCOLS], MM_DT)
            nc.sync.dma_start(out=xs_all, in_=x_d[:, :])
            wth_s = const.tile([128, 2, G5], MM_DT)
            nc.sync.dma_start(out=wth_s, in_=wth_d[:, :, :])
            wsh_s = const.tile([128, 2, G5], MM_DT)
            nc.sync.dma_start(out=wsh_s, in_=wsh_d[:, :, :])
            wc_s = const.tile([128, 2, C], F32)
            nc.sync.dma_start(out=wc_s, in_=wc_d[:, :, :])
            bc_s = const.tile([1, C], F32)
            nc.sync.dma_start(out=bc_s, in_=bc_d[:, :])
            ones_s = const.tile([1, BL], F32)
            nc.vector.memset(ones_s, 1.0)

            # ---- per-stream state (slot 0 stays zero forever)
            h_st = [state.tile([128, 2, SW], MM_DT, name=f"h_st{s}")
                    for s in range(NST)]
            c_st = [state.tile([128, 2, SW], C_DT, name=f"c_st{s}")
                    for s in range(NST)]
            hsum = [state.tile([128, 2, SW], HS_DT, name=f"hsum{s}")
                    for s in range(NST)]

            # (reps>1 repeats the computation via a HW loop, for slope-timing)
            import contextlib
            loop_cm = (tc.For_i(0, reps, 1) if reps > 1
                       else contextlib.nullcontext())
            with loop_cm:
                for s in range(NST):
                    nc.vector.memset(h_st[s], 0.0)
                    nc.gpsimd.memset(c_st[s], 0.0)
                    nc.gpsimd.memset(hsum[s], 0.0)

                for d, tlo, nd, xoff in DIAGS:
                    N = nd * SB
                    ht, hs = tlo * SB, (tlo + 1) * SB

                    # phase 1: GEMMs + gate activations, both streams
                    gt = [None] * NST
                    pss = [None] * NST
                    for s in range(NST):
                        xc = xoff + s * XSTRIDE
                        ps = {}
                        for gname, _, _ in GATES:
                            ps[gname] = psg.tile(
                                [128, 2, 256], F32,
                                name=f"ps{s}_{gname}", tag="ps")
                        # all x-GEMMs first: state-independent, they fill
                        # the PE while the previous tails drain
                        for m in (0, 1):
                            for gname, gc, _ in GATES:
                                mc = gc + m * 128
                                # start=True clears has_written for the
                                # whole (single-bank) tile -> first MM only
                                nc.tensor.matmul(
                                    ps[gname][:, m, 0:N],
                                    wih_s[:, mc:mc + 128],
                                    xs_all[:, xc:xc + N],
                                    start=(m == 0), stop=False)
                        g_s = {}
                        for gname, gc, fn in GATES:
                            for m in (0, 1):
                                mc = gc + m * 128
                                o = ps[gname][:, m, 0:N]
                                nc.tensor.matmul(
                                    o, wth_s[:, 0, mc:mc + 128],
                                    h_st[s][:, 0, ht:ht + N],
                                    start=False, stop=False)
                                nc.tensor.matmul(
                                    o, wth_s[:, 1, mc:mc + 128],
                                    h_st[s][:, 1, ht:ht + N],
                                    start=False, stop=False)
                                nc.tensor.matmul(
                                    o, wsh_s[:, 0, mc:mc + 128],
                                    h_st[s][:, 0, hs:hs + N],
                                    start=False, stop=False)
                                nc.tensor.matmul(
                                    o, wsh_s[:, 1, mc:mc + 128],
                                    h_st[s][:, 1, hs:hs + N],
                                    start=False, stop=True)
                            g = gatep.tile([128, 2, NMAX], GATE_DT,
                                           name=f"g{s}_{gname}",
                                           tag=f"g{s}_{gname}")
                            nc.scalar.activation(out=g[:, :, 0:N],
                                                 in_=ps[gname][:, :, 0:N],
                                                 func=getattr(AF, fn))
                            g_s[gname] = g
                        gt[s] = g_s
                        pss[s] = ps

                    # phase 2: cell update, both streams
                    for s in range(NST):
                        g_s = gt[s]
                        t1 = work.tile([128, 2, NMAX], GATE_DT,
                                       name=f"t1_{s}", tag=f"t1_{s}")
                        nc.vector.tensor_mul(t1[:, :, 0:N],
                                             g_s["i"][:, :, 0:N],
                                             g_s["u"][:, :, 0:N])
                        t2 = work.tile([128, 2, NMAX], C_DT,
                                       name=f"t2_{s}", tag=f"t2_{s}")
                        nc.vector.tensor_mul(t2[:, :, 0:N],
                                             g_s["fs"][:, :, 0:N],
                                             c_st[s][:, :, hs:hs + N])
                        s12 = work.tile([128, 2, NMAX], C_DT,
                                        name=f"s12_{s}", tag=f"s12_{s}")
                        nc.vector.tensor_add(s12[:, :, 0:N], t1[:, :, 0:N],
                                             t2[:, :, 0:N])
                        t3 = work.tile([128, 2, NMAX], C_DT,
                                       name=f"t3_{s}", tag=f"t3_{s}")
                        nc.vector.tensor_mul(t3[:, :, 0:N],
                                             g_s["ft"][:, :, 0:N],
                                             c_st[s][:, :, ht:ht + N])
                        nc.vector.tensor_add(c_st[s][:, :, hs:hs + N],
                                             s12[:, :, 0:N], t3[:, :, 0:N])
                        tcz = work.tile([128, 2, NMAX], GATE_DT,
                                        name=f"tc_{s}", tag=f"tc_{s}")
                        nc.scalar.activation(out=tcz[:, :, 0:N],
                                             in_=c_st[s][:, :, hs:hs + N],
                                             func=AF.Tanh)
                        nc.vector.tensor_mul(h_st[s][:, :, hs:hs + N],
                                             g_s["o"][:, :, 0:N],
                                             tcz[:, :, 0:N])
                        nc.vector.tensor_add(hsum[s][:, :, hs:hs + N],
                                             hsum[s][:, :, hs:hs + N],
                                             h_st[s][:, :, hs:hs + N])

                # ---- mean-pool: fold slots 1..100 onto slot 1, in fp32
                f32acc = work.tile([128, 2, 2, T * SB // 2], F32,
                                   name="f32acc", tag="f32acc")
                for s in range(NST):
                    w = (T // 2) * SB
                    # first fold converts fp16 partials to fp32
                    nc.vector.tensor_add(f32acc[:, s, :, :],
                                         hsum[s][:, :, SB:SB + w],
                                         hsum[s][:, :, SB + w:SB + 2 * w])
                cur = T // 2
                for s in range(NST):
                    acc = f32acc[:, s, :, :]
                    c2 = cur
                    while c2 > 1:
                        if c2 % 2 == 1:
                            last = (c2 - 1) * SB
                            nc.vector.tensor_add(acc[:, :, 0:SB],
                                                 acc[:, :, 0:SB],
                                                 acc[:, :, last:last + SB])
                            c2 -= 1
                        half = c2 // 2
                        w = half * SB
                        nc.vector.tensor_add(acc[:, :, 0:w], acc[:, :, 0:w],
                                             acc[:, :, w:2 * w])
                        c2 = half

                # ---- assemble F (128, 2, 16) = [stream A | stream B]
                fmat = work.tile([128, 2, BL], F32, name="fmat", tag="fmat")
                for s in range(NST):
                    nc.vector.tensor_copy(
                        fmat[:, :, s * SB:(s + 1) * SB],
                        f32acc[:, s, :, 0:SB])

                # ---- classifier: out (BL, C), batch on partitions
                pc = pscls.tile([BL, 512], F32, tag="cls")
                lg = pc[:, 0:C]
                nc.tensor.matmul(lg, fmat[:, 0, :], wc_s[:, 0, :],
                                 start=True, stop=False)
                nc.tensor.matmul(lg, fmat[:, 1, :], wc_s[:, 1, :],
                                 start=False, stop=False)
                nc.tensor.matmul(lg, ones_s[:, :], bc_s[:, :],
                                 start=False, stop=True)

                # ---- log_softmax over free dim
                mx = work.tile([BL, 1], F32, tag="mx")
                nc.vector.reduce_max(out=mx, in_=lg,
                                     axis=mybir.AxisListType.X)
                nmx = work.tile([BL, 1], F32, tag="nmx")
                nc.scalar.mul(out=nmx, in_=mx, mul=-1.0)
                ex = work.tile([BL, C], F32, tag="ex")
                nc.scalar.activation(out=ex, in_=lg, func=AF.Exp, bias=nmx)
                sm = work.tile([BL, 1], F32, tag="sm")
                nc.vector.reduce_sum(out=sm, in_=ex,
                                     axis=mybir.AxisListType.X)
                lse = work.tile([BL, 1], F32, tag="lse")
                nc.scalar.activation(out=lse, in_=sm, func=AF.Ln)
                tot = work.tile([BL, 1], F32, tag="tot")
                nc.vector.tensor_add(tot, mx, lse)
                res = work.tile([BL, C], F32, tag="res")
                nc.vector.tensor_scalar(out=res, in0=lg, scalar1=tot,
                                        scalar2=None,
                                        op0=mybir.AluOpType.subtract)
                nc.sync.dma_start(out=out_d[:, :], in_=res)

    nc.compile()
    return nc


_NC = None


def _get_nc():
    global _NC
    if _NC is None:
        _NC = _build_nc()
    return _NC


def _pack_inputs(data, W_ih, W_th, W_sh, b, weight_c, bias_c):
    """Host-side prep: weights in lhsT layout, x in diagonal-major order."""
    data = np.asarray(data, np.float32)
    # lhsT for the x-GEMM: (I+1, 1280) = [W_ih.T; b] (bias via ones row in x)
    wih = np.concatenate([np.asarray(W_ih, np.float32).T,
                          np.asarray(b, np.float32)[None, :]], 0).astype(MM_NP)
    # lhsT for h-GEMMs: (128, chunk, 1280)
    wth = np.asarray(W_th, np.float32).T.reshape(2, 128, G5).transpose(1, 0, 2)
    wsh = np.asarray(W_sh, np.float32).T.reshape(2, 128, G5).transpose(1, 0, 2)
    wth = np.ascontiguousarray(wth).astype(MM_NP)
    wsh = np.ascontiguousarray(wsh).astype(MM_NP)
    # classifier: fold the 1/(T*J) mean into the weights
    wc = (np.asarray(weight_c, np.float32).T / (T * J)).reshape(2, 128, C)
    wc = np.ascontiguousarray(wc.transpose(1, 0, 2), np.float32)
    bc = np.asarray(bias_c, np.float32)[None, :]

    # x in diagonal-major order per stream: cols (cell-in-diag, batch-row)
    tt = np.concatenate([np.arange(max(0, d - (J - 1)), min(d, T - 1) + 1)
                         for d in range(T + J - 1)])
    jj = np.concatenate([d - np.arange(max(0, d - (J - 1)), min(d, T - 1) + 1)
                         for d in range(T + J - 1)])
    xc = data[tt, jj]                     # (2500, B, I)
    in_maps = []
    for k in range(NCORES):
        blocks = []
        for s in range(NST):
            b0 = k * BL + s * SB
            xk = xc[:, b0:b0 + SB, :]               # (2500, SB, I)
            xk = xk.transpose(2, 0, 1).reshape(I, XSTRIDE)
            blocks.append(xk)
        xk = np.concatenate(blocks, 1)              # (I, XCOLS)
        xdiag = np.concatenate([xk, np.ones((1, XCOLS), np.float32)], 0)
        in_maps.append({
            "xdiag": np.ascontiguousarray(xdiag).astype(MM_NP),
            "wih": wih, "wth": wth, "wsh": wsh, "wc": wc, "bc": bc,
        })
    return in_maps


class _Runner:
    """Persistent jitted SPMD executable (run_bass_via_pjrt traces+jits on
    every call; this caches the jit and keeps inputs device-resident)."""

    def __init__(self, nc):
        import jax
        from jax.sharding import Mesh, PartitionSpec
        from jax.experimental.shard_map import shard_map
        from concourse import mybir as _mb
        from concourse.bass2jax import _bass_exec_p, install_neuronx_cc_hook

        install_neuronx_cc_hook()
        self.nc = nc
        in_names, out_names, out_avals, zero_outs = [], [], [], []
        for alloc in nc.m.functions[0].allocations:
            if not isinstance(alloc, _mb.MemoryLocationSet):
                continue
            name = alloc.memorylocations[0].name
            if alloc.kind == "ExternalInput":
                in_names.append(name)
            elif alloc.kind == "ExternalOutput":
                out_names.append(name)
                shape = tuple(alloc.tensor_shape)
                dtype = _mb.dt.np(alloc.dtype)
                out_avals.append(jax.core.ShapedArray(shape, dtype))
                zero_outs.append(np.zeros(shape, dtype))
        self.in_names, self.out_names = in_names, out_names
        n_params, n_outs = len(in_names), len(out_names)
        all_names = tuple(in_names + out_names)

        def _body(*args):
            return tuple(_bass_exec_p.bind(
                *args, out_avals=tuple(out_avals), in_names=all_names,
                out_names=tuple(out_names), lowering_input_output_aliases=(),
                sim_require_finite=True, sim_require_nnan=True, nc=nc))

        devices = jax.devices()[:NCORES]
        self.mesh = Mesh(np.asarray(devices), ("core",))
        in_specs = (PartitionSpec("core"),) * (n_params + n_outs)
        out_specs = (PartitionSpec("core"),) * n_outs
        self._jit = jax.jit(
            shard_map(_body, mesh=self.mesh, in_specs=in_specs,
                      out_specs=out_specs, check_rep=False),
            donate_argnums=tuple(range(n_params, n_params + n_outs)),
            keep_unused=True)
        self._zeros = zero_outs
        self._dev_in = None

    def put_inputs(self, in_maps):
        import jax
        from jax.sharding import NamedSharding, PartitionSpec
        sh = NamedSharding(self.mesh, PartitionSpec("core"))
        pid = (self.nc.partition_id_tensor.name
               if self.nc.partition_id_tensor else None)
        in_maps = [dict(m) for m in in_maps]
        for k, m in enumerate(in_maps):
            if pid is not None:
                m[pid] = np.array([[k]], dtype=np.uint32)
        self._dev_in = [
            jax.device_put(np.concatenate(
                [np.asarray(m[n]) for m in in_maps], 0), sh)
            for n in self.in_names]

    def run(self):
        zeros = [np.concatenate([z] * NCORES, 0) for z in self._zeros]
        outs = self._jit(*self._dev_in, *zeros)
        return [np.asarray(o) for o in outs]


_RUNNER = None


def _get_runner():
    global _RUNNER
    if _RUNNER is None:
        _RUNNER = _Runner(_get_nc())
    return _RUNNER


def run_on_device(in_maps):
    r = _get_runner()
    r.put_inputs(in_maps)
    out = r.run()[0]          # (8*BL, C) concat over cores
    return out.reshape(NCORES * BL, C)


def kernel(data, W_ih, W_th, W_sh, b, weight_c, bias_c, batch_size=None,
           **_ignored):
    in_maps = _pack_inputs(data, W_ih, W_th, W_sh, b, weight_c, bias_c)
    return run_on_device(in_maps)


if __name__ == "__main__":
    d = np.load(os.path.join(os.path.dirname(__file__), "inputs.npz"))
    out = kernel(d["data"], d["W_ih"], d["W_th"], d["W_sh"], d["b"],
                 d["weight_c"], d["bias_c"])
    exp = np.load(os.path.join(os.path.dirname(__file__), "oracle64.npy"))
    aerr = np.abs(out - exp).max()
    print("absmax err vs fp64 oracle:", aerr,
          " rel:", aerr / np.abs(exp).max())
